# revision 1
# baseline (speedup 1.0000x reference)
"""Trainium2 Bass kernel for ConcatAtten (additive / Bahdanau-style attention).

Reference computation (all fp32):
    q = query @ W1                      # [B, TQ, E]
    k = key   @ W2                      # [B, TK, E]
    sjt[b,t,q] = sum_e tanh(k[b,t,e] + q[b,q,e]) * vc[e]   # [B, TK, TQ]
    attens = softmax(sjt, axis=2)       # over TQ
    out = value @ attens                # [B, D, TK] @ [B, TK, TQ] -> [B, D, TQ]

Sharding: 8 cores = 4 batches x 2 TK-halves. Each core gets query[b] (full),
key[b, half], value[b][:, half], computes a partial out[b] over its 256 key
rows; the host sums the two halves (softmax rows are per-t, so the split is
exact; only the final contraction over t needs the cross-core add).

Per-core dataflow (e on partitions for the tanh stage):
  - PE transposes query/key/value blocks; projections give
      qproj[e=128, q=512], kproj[e=128, t=256] in SBUF.
  - DVE tensor_scalar_add builds tanh inputs: in[e, t, q] = qproj[e,q] + kproj[e,t]
    (one [128, 512] op per t — fp32 tensor_scalar runs in 2x mode).
  - ACT runs one big in-place Tanh per 32-t sub-block ([128, 32*512]).
  - PE reduces over e with a sliding-window weight: wsel[e, 128] = vc[e],
    zeros elsewhere; lhsT = wsel[:, 128-t : 256-t] puts vc in weight column t,
    so matmul accumulates s[t, q] directly into a [t=128, q=512] PSUM tile
    (all other rows get +0).
  - ACT Exp with accum_out gives exp(s) and its row-sum in one pass; the
    reciprocal sum is folded into value^T, and 4 final matmuls produce the
    [D=256, TQ=512] partial output.

Engine instructions carry only ONE semaphore wait slot; building with
bacc.Bacc (whose generate_event_semaphores pass splits multi-waits into
event-semaphore instructions) is required — plain bass.Bass fails codegen.
Score/output matmuls use float32r (fp32 bits, relaxed-precision matmul at
1 cycle/column vs 4 for strict fp32). Sub-block sizes ramp up/down at the
kernel boundaries and the work pool is triple-buffered so DVE adds, ACT
tanh, and PE matmuls pipeline; the cost-model timeline predicts ~138 us
per core with ACT (the tanh floor) at ~86% occupancy.
"""

import numpy as np

B, TQ, TK = 4, 512, 512
E, F, D = 128, 256, 256  # E, TWO_E (=F), and value depth D (=TWO_E)
TKS = TK // 2            # per-core key rows
TB = 8                   # t sub-block size for tanh tiles
NBLK = TKS // 128        # 128-t PSUM blocks per core
NSUB = 128 // TB         # sub-blocks per PSUM block

_CACHE = {}


def _build_nc():
    import concourse.bass as bass
    import concourse.tile as tile
    from concourse import bacc, mybir
    from concourse.masks import make_identity

    f32 = mybir.dt.float32
    f32r = mybir.dt.float32r  # fp32 bits, relaxed-precision matmul at 1 cyc/col
    Tanh = mybir.ActivationFunctionType.Tanh
    Exp = mybir.ActivationFunctionType.Exp

    nc = bacc.Bacc(None, target_bir_lowering=False)
    q_d = nc.dram_tensor("q", [TQ, F], f32, kind="ExternalInput")
    k_d = nc.dram_tensor("k", [TKS, F], f32, kind="ExternalInput")
    v_d = nc.dram_tensor("v", [D, TKS], f32, kind="ExternalInput")
    w1_d = nc.dram_tensor("w1", [F, E], f32, kind="ExternalInput")
    w2_d = nc.dram_tensor("w2", [F, E], f32, kind="ExternalInput")
    ws_d = nc.dram_tensor("wsel", [E, 2 * E], f32r, kind="ExternalInput")
    out_d = nc.dram_tensor("out", [D, TQ], f32, kind="ExternalOutput")

    with tile.TileContext(nc) as tc:
        with (
            tc.tile_pool(name="cst", bufs=1) as cst,
            tc.tile_pool(name="work", bufs=3) as work,
            tc.tile_pool(name="ps_t", bufs=2, space="PSUM") as ps_t,
            tc.tile_pool(name="ps_qp", bufs=1, space="PSUM") as ps_qp,
            tc.tile_pool(name="ps_kp", bufs=1, space="PSUM") as ps_kp,
            tc.tile_pool(name="ps_s", bufs=2, space="PSUM") as ps_s,
            tc.tile_pool(name="ps_o", bufs=1, space="PSUM") as ps_o,
        ):
            # ---- load inputs (natural layouts, contiguous per partition) ----
            k_sb = cst.tile([128, TKS // 128, F], f32)     # [p, ti, f]
            k_r = k_d.rearrange("(i p) f -> p i f", p=128)
            for i in range(TKS // 128):
                nc.sync.dma_start(out=k_sb[:, i, :], in_=k_r[:, i, :])
            q_sb = cst.tile([128, TQ // 128, F], f32)      # [p, qi, f]
            q_r = q_d.rearrange("(i p) f -> p i f", p=128)
            for i in range(TQ // 128):
                nc.sync.dma_start(out=q_sb[:, i, :], in_=q_r[:, i, :])
            v_sb = cst.tile([128, D // 128, TKS], f32)     # [p, di, t]
            nc.sync.dma_start(out=v_sb, in_=v_d.rearrange("(i p) t -> p i t", p=128))
            w1_sb = cst.tile([128, F // 128, E], f32)      # [p, fi, e]
            nc.sync.dma_start(out=w1_sb, in_=w1_d.rearrange("(i p) e -> p i e", p=128))
            w2_sb = cst.tile([128, F // 128, E], f32)
            nc.sync.dma_start(out=w2_sb, in_=w2_d.rearrange("(i p) e -> p i e", p=128))
            wsel = cst.tile([128, 2 * E], f32r)
            nc.sync.dma_start(out=wsel, in_=ws_d[:, :])

            ident = cst.tile([128, 128], f32)
            make_identity(nc, ident)

            # ---- transposes + projections: key first so kproj lands early ----
            kTf = cst.tile([128, F // 128, TKS], f32)      # [f, fj, t]
            for i in range(TKS // 128):
                for j in range(F // 128):
                    tp = ps_t.tile([128, 128], f32, tag="tp", name="tp")
                    nc.tensor.transpose(tp, k_sb[:, i, j * 128:(j + 1) * 128], ident)
                    nc.vector.tensor_copy(out=kTf[:, j, i * 128:(i + 1) * 128], in_=tp)
            kp_ps = ps_kp.tile([128, TKS], f32)
            for j in range(F // 128):
                nc.tensor.matmul(kp_ps, lhsT=w2_sb[:, j, :], rhs=kTf[:, j, :],
                                 start=(j == 0), stop=(j == F // 128 - 1))
            kproj = cst.tile([128, TKS], f32)
            nc.vector.tensor_copy(out=kproj, in_=kp_ps)

            qTf = cst.tile([128, F // 128, TQ], f32)       # [f, fj, q]
            for i in range(TQ // 128):
                for j in range(F // 128):
                    tp = ps_t.tile([128, 128], f32, tag="tp", name="tp")
                    nc.tensor.transpose(tp, q_sb[:, i, j * 128:(j + 1) * 128], ident)
                    nc.vector.tensor_copy(out=qTf[:, j, i * 128:(i + 1) * 128], in_=tp)
            qp_ps = ps_qp.tile([128, TQ], f32)
            for j in range(F // 128):
                nc.tensor.matmul(qp_ps, lhsT=w1_sb[:, j, :], rhs=qTf[:, j, :],
                                 start=(j == 0), stop=(j == F // 128 - 1))
            qproj = cst.tile([128, TQ], f32)
            nc.vector.tensor_copy(out=qproj, in_=qp_ps)

            vT = cst.tile([128, NBLK, D], f32)             # [t, tj, d]
            for i in range(D // 128):
                for j in range(NBLK):
                    tp = ps_t.tile([128, 128], f32, tag="tp", name="tp")
                    nc.tensor.transpose(tp, v_sb[:, i, j * 128:(j + 1) * 128], ident)
                    nc.vector.tensor_copy(out=vT[:, j, i * 128:(i + 1) * 128], in_=tp)


            exp_t = cst.tile([128, NBLK, TQ], f32r)        # exp(s), per t-block
            sums = cst.tile([128, NBLK], f32)
            rsum = cst.tile([128, NBLK], f32)
            vscaled = cst.tile([128, NBLK, D], f32r)

            def finish_block(blk, s_ps):
                nc.scalar.activation(out=exp_t[:, blk, :], in_=s_ps, func=Exp,
                                     accum_out=sums[:, blk:blk + 1])
                nc.vector.reciprocal(out=rsum[:, blk:blk + 1], in_=sums[:, blk:blk + 1])
                nc.vector.tensor_scalar_mul(out=vscaled[:, blk, :], in0=vT[:, blk, :],
                                            scalar1=rsum[:, blk:blk + 1])

            # Sub-block sizes ramp up at kernel start (prime the pipeline
            # sooner) and down at the end (shrink the post-tanh tail).
            ramp_up = [4, 4] + [8] * 15
            ramp_dn = list(reversed(ramp_up))
            segs = []
            for blk in range(NBLK):
                if blk == 0:
                    sizes = ramp_up
                elif blk == NBLK - 1:
                    sizes = ramp_dn
                else:
                    sizes = [TB] * NSUB
                t0 = 0
                for tb in sizes:
                    segs.append((blk, t0, tb))
                    t0 += tb
                assert t0 == 128

            s_tiles = [None] * NBLK
            for blk, t0, tb in segs:
                if t0 == 0:
                    s_tiles[blk] = ps_s.tile([128, TQ], f32, tag="s_ps", name="s_ps")
                it_in = work.tile([128, tb, TQ], f32, tag="it_in", name="it_in")
                for i in range(tb):
                    t = blk * 128 + t0 + i
                    nc.vector.tensor_scalar_add(out=it_in[:, i, :], in0=qproj,
                                                scalar1=kproj[:, t:t + 1])
                it_out = work.tile([128, tb, TQ], f32r, tag="it_out", name="it_out")
                nc.scalar.activation(out=it_out, in_=it_in, func=Tanh)
                for i in range(tb):
                    tl = t0 + i
                    nc.tensor.matmul(s_tiles[blk],
                                     lhsT=wsel[:, 128 - tl:256 - tl],
                                     rhs=it_out[:, i, :],
                                     start=(tl == 0), stop=(tl == 127))
                # Emit blk's softmax one sub-block into the NEXT blk so ACT
                # doesn't stall waiting on blk's last PE matmuls.
                if t0 == 0 and blk > 0:
                    finish_block(blk - 1, s_tiles[blk - 1])
            finish_block(NBLK - 1, s_tiles[NBLK - 1])

            # ---- output: out[d, q] = sum_t vscaled[t, d] * exp[t, q] ----
            for i in range(D // 128):
                o_ps = ps_o.tile([128, TQ], f32, tag="o_ps", name="o_ps")
                for j in range(NBLK):
                    nc.tensor.matmul(o_ps,
                                     lhsT=vscaled[:, j, i * 128:(i + 1) * 128],
                                     rhs=exp_t[:, j, :],
                                     start=(j == 0), stop=(j == NBLK - 1))
                o_sb = work.tile([128, TQ], f32, tag="osb", name="osb")
                nc.vector.tensor_copy(out=o_sb, in_=o_ps)
                nc.sync.dma_start(out=out_d[i * 128:(i + 1) * 128, :], in_=o_sb)
    nc.finalize()
    return nc


def kernel(query, key, value, W1, W2, vc):
    from concourse.bass_utils import run_bass_kernel_spmd

    query = np.ascontiguousarray(np.asarray(query, dtype=np.float32))
    key = np.ascontiguousarray(np.asarray(key, dtype=np.float32))
    value = np.ascontiguousarray(np.asarray(value, dtype=np.float32))
    W1 = np.ascontiguousarray(np.asarray(W1, dtype=np.float32))
    W2 = np.ascontiguousarray(np.asarray(W2, dtype=np.float32))
    vc = np.asarray(vc, dtype=np.float32)

    wsel = np.zeros((E, 2 * E), dtype=np.float32)
    wsel[:, 128] = vc

    if "nc" not in _CACHE:
        _CACHE["nc"] = _build_nc()
    nc = _CACHE["nc"]

    in_maps = []
    for c in range(8):
        b, h = divmod(c, 2)
        in_maps.append({
            "q": query[b],
            "k": np.ascontiguousarray(key[b, h * TKS:(h + 1) * TKS, :]),
            "v": np.ascontiguousarray(value[b][:, h * TKS:(h + 1) * TKS]),
            "w1": W1,
            "w2": W2,
            "wsel": wsel,
        })

    res = run_bass_kernel_spmd(nc, in_maps, core_ids=list(range(8)))
    parts = [r["out"] for r in res.results]
    out = np.empty((B, D, TQ), dtype=np.float32)
    for b in range(B):
        out[b] = parts[2 * b] + parts[2 * b + 1]
    return out

